# revision 26
# baseline (speedup 1.0000x reference)
"""KAN layer (LayerNorm -> RBF-spline + base linear) on 8 Trainium2 cores.

Math: the reference reduces to
    xn = LayerNorm(x) * ln_w + ln_b                       (B, D)
    S  = sum_j exp(-beta * (xn - g_j)^2)                  (B, D)
    out = xn @ scale_base.T + S @ Wd.T + bias             (B, O)
with Wd = spline_weight.sum(-1).

RBF evaluation: for a uniform grid whose spacing dg is small relative to the
Gaussian width (beta*dg^2 << pi^2), the comb sum is spectrally close to its
midpoint-rule integral, i.e. a difference of two erfs:
    S(t) ~= K * (erf(s*(t-a)) - erf(s*(t-b)))
The four parameters (K, s, a, b) are fitted on the host at build time
(max fit error ~5e-5 for beta=2, G=8 on [-1,1]); K is folded into Wd during
weight prep so the device evaluates S with just TWO scalar-engine Erf
activations and one vector subtract.

Distribution (8 cores, two launches):
  Phase 1 (out-dim sharded weights, batch-sharded activations): core i
    (a) sums its spline_weight slice over G on the vector engine,
        PE-transposes the [scale_base | K*Wd] blocks and evicts them as
        bf16 C.T panels ct[ob][d_inner][kb][o];
    (b) LayerNorms its 512 batch rows, evaluates S with the two-erf form,
        and PE-transposes xn/S blocks into a bf16 A.T written to DRAM.
    Both halves hide under the ~21 MB/core spline-weight DMA stream.
  Host concatenates the ct slices -> full C.T (bf16).
  Phase 2 (batch sharded): pure matmul: core i loads its A.T and streams
    the 16 C.T panels; out.T[ob] = sum_kb ct[ob,:,kb,:].T @ A.T[:,kb,:]
    with bf16 matmuls accumulating in fp32 PSUM; bias is fused into the
    PSUM eviction.  Host transposes/concats the 8 out.T slices.

When ln_weight == 1 and ln_bias == 0 (checked at runtime against the actual
inputs), the normalized rows are used directly and the two broadcast-affine
vector passes are skipped; a general variant handles arbitrary ln params.
Inputs outside the fast path (non-uniform grid, bad erf fit, or unexpected
shapes) fall back to an exact numpy evaluation.
"""

import sys

if "/opt/trn_rl_repo" not in sys.path:
    sys.path.insert(0, "/opt/trn_rl_repo")

import numpy as np

import concourse.bass as bass  # noqa: F401  (registers engine methods)
import concourse.mybir as mybir
from concourse import bacc
from concourse.bass_utils import run_bass_kernel_spmd
from concourse.masks import make_identity
from concourse.tile import TileContext

dt = mybir.dt
AF = mybir.ActivationFunctionType
OP = mybir.AluOpType

N_CORES = 8
P = 128
B = 4096
D = 2048          # in_dim
O = 2048          # out_dim
G = 8
B_SH = B // N_CORES      # 512 batch rows per core (phase 2)
O_SH = O // N_CORES      # 256 out rows per core (phase 1)
KB = (2 * D) // P        # 32 contraction blocks (xn + S stacked)
OB = O // P              # 16 output row-blocks
OB_SH = O_SH // P        # 2 output row-blocks per core in phase 1
DB = D // P              # 16 blocks along D
LN_EPS = 1e-5

_COMPILED = {}


def _fit_erf_params(beta, g0, dg, n_grid):
    """Fit S(t) = K*(erf(s*(t-a)) - erf(s*(t-b))). Returns params + max err."""
    from math import pi, sqrt

    from scipy.optimize import minimize
    from scipy.special import erf as serf

    grid = g0 + dg * np.arange(n_grid)
    t = np.linspace(g0 - 8.0, grid[-1] + 8.0, 30001)
    s_exact = np.zeros_like(t)
    for g in grid:
        s_exact += np.exp(-beta * (t - g) ** 2)
    k0 = sqrt(pi / beta) / (2 * dg)
    a0, b0 = g0 - dg / 2, grid[-1] + dg / 2

    def loss(p):
        k_, s_, a_, b_ = p
        return np.abs(k_ * (serf(s_ * (t - a_)) - serf(s_ * (t - b_))) - s_exact).max()

    res = minimize(loss, [k0, sqrt(beta), a0, b0], method="Nelder-Mead",
                   options={"xatol": 1e-12, "fatol": 1e-14, "maxiter": 6000})
    k_, s_, a_, b_ = (float(v) for v in res.x)
    return k_, s_, a_, b_, float(res.fun)


def _build_phase1(k_scale, s_erf, be1, be2, unit_ln):
    nc = bacc.Bacc("TRN2", target_bir_lowering=False, debug=False,
                   num_devices=N_CORES)
    w = nc.dram_tensor("w", [O_SH, D, G], dt.float32, kind="ExternalInput")
    sb = nc.dram_tensor("sb", [O_SH, D], dt.float32, kind="ExternalInput")
    x = nc.dram_tensor("x", [B_SH, D], dt.float32, kind="ExternalInput")
    lnw = nc.dram_tensor("lnw", [D], dt.float32, kind="ExternalInput")
    lnb = nc.dram_tensor("lnb", [D], dt.float32, kind="ExternalInput")
    # ct[ot][d_inner][kb][o_inner]: C.T blocks; kb 0..DB-1 = scale_base.T,
    # kb DB..KB-1 = (K*Wd).T.  Per-partition contiguous for phase 2 reads.
    ct = nc.dram_tensor("ct", [OB_SH, P, KB, P], dt.bfloat16,
                        kind="ExternalOutput")
    # A.T for this core's batch rows: [d_inner][kb][b]
    at_d = nc.dram_tensor("at", [P, KB, B_SH], dt.bfloat16,
                          kind="ExternalOutput")

    ic_n = 4
    chunk = D // ic_n  # 512
    n_bt = B_SH // P   # 4 batch tiles
    wp_bufs = 3 if unit_ln else 2
    with TileContext(nc) as tc:
        with (
            tc.tile_pool(name="sbuf", bufs=2) as sbuf,
            tc.tile_pool(name="wpool", bufs=wp_bufs) as wpool,
            tc.tile_pool(name="stg", bufs=2) as stg,
            tc.tile_pool(name="xp", bufs=1) as xp,
            tc.tile_pool(name="ew", bufs=2) as ew,
            tc.tile_pool(name="bw", bufs=2) as bw,
            tc.tile_pool(name="st", bufs=2) as st,
            tc.tile_pool(name="atp", bufs=1) as atp,
            tc.tile_pool(name="const", bufs=1) as const,
            tc.tile_pool(name="psum", bufs=2, space="PSUM") as psum,
            tc.tile_pool(name="pst", bufs=2, space="PSUM") as pst,
        ):
            ident = const.tile([P, P], dt.float32)
            make_identity(nc, ident[:])
            ident_bf = const.tile([P, P], dt.bfloat16)
            make_identity(nc, ident_bf[:])
            eps_t = const.tile([P, 1], dt.float32)
            nc.vector.memset(eps_t[:], LN_EPS)
            be1_t = const.tile([P, 1], dt.float32)
            nc.vector.memset(be1_t[:], float(be1))
            be2_t = const.tile([P, 1], dt.float32)
            nc.vector.memset(be2_t[:], float(be2))
            if not unit_ln:
                wt_b = const.tile([P, D], dt.float32)
                nc.sync.dma_start(wt_b[:1, :], lnw.ap()[None, :])
                nc.gpsimd.partition_broadcast(wt_b[:], wt_b[:1, :])
                bt_b = const.tile([P, D], dt.float32)
                nc.sync.dma_start(bt_b[:1, :], lnb.ap()[None, :])
                nc.gpsimd.partition_broadcast(bt_b[:], bt_b[:1, :])

            at = atp.tile([P, KB, B_SH], dt.bfloat16)

            # ---- all DMA loads issued upfront ----
            # x on the gpsimd software-DGE queue; weights own the sync queue.
            xts = []
            for bt_i in range(n_bt):
                xt = xp.tile([P, D], dt.float32, tag=f"x{bt_i}")
                nc.gpsimd.dma_start(xt[:], x.ap()[bt_i * P:(bt_i + 1) * P, :])
                xts.append(xt)
            sbts = []
            for ot in range(OB_SH):
                sbt = sbuf.tile([P, D], dt.float32, tag="sbt")
                nc.sync.dma_start(sbt[:], sb.ap()[ot * P:(ot + 1) * P, :])
                sbts.append(sbt)
            wdts = [sbuf.tile([P, D], dt.float32, tag="wdt",
                              name=f"wdt{i}") for i in range(OB_SH)]
            stages = [stg.tile([P, KB, P], dt.bfloat16, tag="stage",
                               name=f"stage{i}") for i in range(OB_SH)]
            wtiles = []
            for ot in range(OB_SH):
                for ic in range(ic_n):
                    wt_ = wpool.tile([P, chunk, G], dt.float32, tag="wt")
                    nc.sync.dma_start(
                        wt_[:],
                        w.ap()[ot * P:(ot + 1) * P,
                               ic * chunk:(ic + 1) * chunk, :])
                    wtiles.append((ot, ic, wt_))

            def reduce_chunk(idx):
                ot, ic, wt_ = wtiles[idx]
                nc.vector.reduce_sum(
                    wdts[ot][:, ic * chunk:(ic + 1) * chunk],
                    wt_[:], axis=mybir.AxisListType.X)

            def sb_transposes(ot):
                for grp in range(DB // 4):
                    kb0 = grp * 4
                    pt = psum.tile([P, 4, P], dt.float32, tag="pt")
                    for j in range(4):
                        kb = kb0 + j
                        nc.tensor.transpose(pt[:, j],
                                            sbts[ot][:, kb * P:(kb + 1) * P],
                                            ident[:])
                    nc.scalar.copy(stages[ot][:, kb0:kb0 + 4], pt[:])

            def wd_transposes(ot):
                for grp in range(DB // 4):
                    kb0 = grp * 4
                    pt2 = psum.tile([P, 4, P], dt.float32, tag="pt2")
                    for j in range(4):
                        kb = kb0 + j
                        nc.tensor.transpose(pt2[:, j],
                                            wdts[ot][:, kb * P:(kb + 1) * P],
                                            ident[:])
                    nc.vector.tensor_scalar_mul(
                        stages[ot][:, DB + kb0:DB + kb0 + 4], pt2[:],
                        float(k_scale))

            # sb0 transposes first: data lands early, warms the PE.
            sb_transposes(0)

            # ---- batch tiles, with ot0 weight-chunk reduces interleaved so
            # the DVE drains the wpool ring while scalar runs the erf chain.
            for bt_i in range(n_bt):
                xt = xts[bt_i]
                st6 = st.tile([P, 4, 6], dt.float32, tag="st6")
                for i in range(4):
                    nc.vector.bn_stats(st6[:, i, :],
                                       xt[:, i * 512:(i + 1) * 512])
                mv = st.tile([P, 2], dt.float32, tag="mv")
                nc.vector.bn_aggr(mv[:], st6[:])
                sd = st.tile([P, 1], dt.float32, tag="sd")
                nc.scalar.activation(sd[:], mv[:, 1:2], AF.Sqrt,
                                     bias=eps_t[:])
                istd = st.tile([P, 1], dt.float32, tag="istd")
                nc.vector.reciprocal(istd[:], sd[:])
                bz = st.tile([P, 1], dt.float32, tag="bz")
                nc.vector.tensor_scalar(bz[:], mv[:, 0:1], istd[:], -1.0,
                                        OP.mult, OP.mult)

                xn_bf = bw.tile([P, D], dt.bfloat16, tag="xn")
                if unit_ln:
                    nc.scalar.activation(xn_bf[:], xt[:], AF.Identity,
                                         scale=istd[:], bias=bz[:])
                else:
                    scr = ew.tile([P, D], dt.float32, tag="scr")
                    nc.scalar.activation(scr[:], xt[:], AF.Identity,
                                         scale=istd[:], bias=bz[:])
                    nc.vector.tensor_mul(scr[:], scr[:], wt_b[:])
                    nc.vector.tensor_add(xn_bf[:], scr[:], bt_b[:])

                e1 = bw.tile([P, D], dt.bfloat16, tag="e1")
                nc.scalar.activation(e1[:], xn_bf[:], AF.Erf,
                                     scale=float(s_erf), bias=be1_t[:])
                e2 = bw.tile([P, D], dt.bfloat16, tag="e2")
                nc.scalar.activation(e2[:], xn_bf[:], AF.Erf,
                                     scale=float(s_erf), bias=be2_t[:])
                s_bf = bw.tile([P, D], dt.bfloat16, tag="s")
                nc.vector.tensor_sub(s_bf[:], e1[:], e2[:])

                for grp in range(DB // 4):
                    kb0 = grp * 4
                    ptx = pst.tile([P, 4, P], dt.bfloat16, tag="ptx")
                    pts = pst.tile([P, 4, P], dt.bfloat16, tag="pts")
                    for j in range(4):
                        kb = kb0 + j
                        nc.tensor.transpose(ptx[:, j],
                                            xn_bf[:, kb * P:(kb + 1) * P],
                                            ident_bf[:])
                        nc.tensor.transpose(pts[:, j],
                                            s_bf[:, kb * P:(kb + 1) * P],
                                            ident_bf[:])
                    dst_x = at[:, kb0:kb0 + 4, bt_i * P:(bt_i + 1) * P]
                    dst_s = at[:, DB + kb0:DB + kb0 + 4,
                               bt_i * P:(bt_i + 1) * P]
                    if grp % 2 == 0:
                        nc.scalar.copy(dst_x, ptx[:])
                        nc.vector.tensor_copy(dst_s, pts[:])
                    else:
                        nc.vector.tensor_copy(dst_x, ptx[:])
                        nc.scalar.copy(dst_s, pts[:])

                reduce_chunk(bt_i)  # ot0 chunk bt_i

            nc.gpsimd.dma_start(at_d.ap()[:], at[:])

            # ---- finish ot0 ----
            wd_transposes(0)
            nc.sync.dma_start(ct.ap()[0], stages[0][:])

            # ---- ot1 ----
            for ic in range(ic_n):
                reduce_chunk(ic_n + ic)
            sb_transposes(1)
            wd_transposes(1)
            nc.sync.dma_start(ct.ap()[1], stages[1][:])
    nc.compile()
    return nc


def _build_phase2():
    nc = bacc.Bacc("TRN2", target_bir_lowering=False, debug=False,
                   num_devices=N_CORES)
    bias = nc.dram_tensor("bias", [O], dt.float32, kind="ExternalInput")
    ct = nc.dram_tensor("ct", [OB, P, KB, P], dt.bfloat16,
                        kind="ExternalInput")
    at_d = nc.dram_tensor("at", [P, KB, B_SH], dt.bfloat16,
                          kind="ExternalInput")
    oy = nc.dram_tensor("oy", [O, B_SH], dt.float32, kind="ExternalOutput")

    with TileContext(nc) as tc:
        with (
            tc.tile_pool(name="const", bufs=1) as const,
            tc.tile_pool(name="at", bufs=1) as atp,
            tc.tile_pool(name="ctp", bufs=6) as ctp,
            tc.tile_pool(name="outp", bufs=3) as outp,
            tc.tile_pool(name="psm", bufs=6, space="PSUM") as psm,
        ):
            bias_t = const.tile([P, OB], dt.float32)
            nc.sync.dma_start(bias_t[:],
                              bias.ap().rearrange("(ob p) -> p ob", p=P))
            # Load A.T in kb-chunks interleaved with the first C.T panels so
            # the first accumulation group starts as early as possible.
            at = atp.tile([P, KB, B_SH], dt.bfloat16)
            n_ch = 4

            panels = []
            for ob in range(n_ch):
                ck = KB // n_ch
                nc.sync.dma_start(at[:, ob * ck:(ob + 1) * ck, :],
                                  at_d.ap()[:, ob * ck:(ob + 1) * ck, :])
                panel = ctp.tile([P, KB, P], dt.bfloat16, tag="panel")
                nc.sync.dma_start(panel[:], ct.ap()[ob])
                panels.append(panel)

            for ob in range(OB):
                if ob < n_ch:
                    panel = panels[ob]
                else:
                    panel = ctp.tile([P, KB, P], dt.bfloat16, tag="panel")
                    nc.sync.dma_start(panel[:], ct.ap()[ob])
                ps = psm.tile([P, B_SH], dt.float32, tag="mm")
                for kb in range(KB):
                    nc.tensor.matmul(ps[:], panel[:, kb], at[:, kb],
                                     start=(kb == 0), stop=(kb == KB - 1))
                ot_s = outp.tile([P, B_SH], dt.float32, tag="osb")
                if ob % 2 == 0:
                    nc.scalar.activation(ot_s[:], ps[:], AF.Identity,
                                         bias=bias_t[:, ob:ob + 1])
                    nc.sync.dma_start(oy.ap()[ob * P:(ob + 1) * P, :],
                                      ot_s[:])
                else:
                    nc.vector.tensor_scalar_add(ot_s[:], ps[:],
                                                bias_t[:, ob:ob + 1])
                    nc.scalar.dma_start(oy.ap()[ob * P:(ob + 1) * P, :],
                                        ot_s[:])
    nc.compile()
    return nc


def _get_phase1(k_scale, s_erf, be1, be2, unit_ln):
    key = ("p1", round(float(k_scale), 9), round(float(s_erf), 9),
           round(float(be1), 9), round(float(be2), 9), bool(unit_ln))
    if key not in _COMPILED:
        _COMPILED[key] = _build_phase1(k_scale, s_erf, be1, be2, unit_ln)
    return _COMPILED[key]


def _get_phase2():
    if "p2" not in _COMPILED:
        _COMPILED["p2"] = _build_phase2()
    return _COMPILED["p2"]


def _kernel_numpy(x, ln_weight, ln_bias, spline_weight, scale_base, bias,
                  rbf_beta, grid):
    """Exact fallback for inputs outside the compiled fast path."""
    x = np.asarray(x, np.float64)
    mu = x.mean(-1, keepdims=True)
    var = ((x - mu) ** 2).mean(-1, keepdims=True)
    xn = (x - mu) / np.sqrt(var + LN_EPS) * np.asarray(ln_weight, np.float64) \
        + np.asarray(ln_bias, np.float64)
    beta = float(np.clip(np.asarray(rbf_beta, np.float64).reshape(-1)[0],
                         0.5, 6.0))
    g = np.asarray(grid, np.float64).reshape(-1)
    s = np.zeros_like(xn)
    for gj in g:
        s += np.exp(-beta * np.square(xn - gj))
    wd = np.asarray(spline_weight, np.float64).sum(-1)
    out = xn @ np.asarray(scale_base, np.float64).T + s @ wd.T \
        + np.asarray(bias, np.float64)
    return out.astype(np.float32)


def erf_params(rbf_beta, grid):
    """(K, s, a, b, fit_err) for the two-erf S form, or None if unusable."""
    beta = float(np.clip(np.asarray(rbf_beta, np.float64).reshape(-1)[0],
                         0.5, 6.0))
    grid_f = np.asarray(grid, np.float64).reshape(-1)
    if len(grid_f) < 2:
        return None
    diffs = np.diff(grid_f)
    dg = float(diffs.mean())
    if dg <= 0 or np.max(np.abs(diffs - dg)) > 1e-5 * max(abs(dg), 1e-30):
        return None
    try:
        k_, s_, a_, b_, err = _fit_erf_params(beta, float(grid_f[0]), dg,
                                              len(grid_f))
    except Exception:
        return None
    if err > 2e-3:
        return None
    return k_, s_, a_, b_, err


def kernel(x, ln_weight, ln_bias, spline_weight, scale_base, bias, rbf_beta,
           grid):
    x = np.ascontiguousarray(np.asarray(x, dtype=np.float32))
    ln_weight = np.asarray(ln_weight, dtype=np.float32)
    ln_bias = np.asarray(ln_bias, dtype=np.float32)
    spline_weight = np.asarray(spline_weight, dtype=np.float32)
    scale_base = np.asarray(scale_base, dtype=np.float32)
    bias = np.asarray(bias, dtype=np.float32)

    fast = (x.shape == (B, D) and spline_weight.shape == (O, D, G)
            and scale_base.shape == (O, D))
    params = erf_params(rbf_beta, grid) if fast else None
    if params is None:
        return _kernel_numpy(x, ln_weight, ln_bias, spline_weight, scale_base,
                             bias, rbf_beta, grid)
    k_, s_, a_, b_, _ = params
    unit_ln = bool(np.all(ln_weight == 1.0) and np.all(ln_bias == 0.0))

    # ---- phase 1: weight prep + LN/erf/A.T (hidden under weight DMA) ----
    nc1 = _get_phase1(k_, s_, -s_ * a_, -s_ * b_, unit_ln)
    in1 = [{
        "w": np.ascontiguousarray(spline_weight[i * O_SH:(i + 1) * O_SH]),
        "sb": np.ascontiguousarray(scale_base[i * O_SH:(i + 1) * O_SH]),
        "x": np.ascontiguousarray(x[i * B_SH:(i + 1) * B_SH]),
        "lnw": ln_weight, "lnb": ln_bias,
    } for i in range(N_CORES)]
    res1 = run_bass_kernel_spmd(nc1, in1, core_ids=list(range(N_CORES)))
    ct_full = np.ascontiguousarray(
        np.concatenate([res1.results[i]["ct"] for i in range(N_CORES)],
                       axis=0))  # (OB, P, KB, P) bf16

    # ---- phase 2: pure matmul (batch sharded) ----
    nc2 = _get_phase2()
    in2 = [{
        "bias": bias, "ct": ct_full, "at": res1.results[i]["at"],
    } for i in range(N_CORES)]
    res2 = run_bass_kernel_spmd(nc2, in2, core_ids=list(range(N_CORES)))

    out = np.empty((B, O), dtype=np.float32)
    for i in range(N_CORES):
        out[i * B_SH:(i + 1) * B_SH, :] = res2.results[i]["oy"].T
    return out
